# revision 22
# baseline (speedup 1.0000x reference)
"""Trainium2 Bass kernel for nn_Attention_12137577578573 (v4).

Full multi-head attention (QKV projection + masked softmax + context) for
B=4, F=T=2048, CF=CT=1024, H=16, DH=64, sharded over 8 NeuronCores as
(batch b, head-group hg): core i = (b = i // 2, hg = i % 2), each core
computing 1 batch x 8 heads.

Layout strategy (contraction dim on partitions):
  - host pre-transposes from/to tensors -> xT/yT [C, F]; yT and K^T live
    as per-512-column-chunk tiles so early chains don't dep-wait on whole
    tensors (Tile deps are tile-granular).
  - Q^T, K^T in transposed layout [cols, F]/[cols, T]; 2 heads per
    128-partition tile.
  - scores S^T [T, F] (T on partitions); softmax denominator via a
    ones-column appended to V; bv folded out on the host (context bias
    is additive); mask applied as P = exp(alpha*S) * maskT.
  - exp emission per tt ([128,1024] PSUM -> bf16 SBUF); mask multiplies
    are emitted one unit LATE (during unit u+1) so their DMA-wait can't
    head-of-line-block the DVE FIFO behind projection bias-adds.
  - context of unit u-1 runs tt-outer, 2 t-tiles per slot in slots 1-8 of
    unit u (consumes v[tt] incrementally, frees pT(u-1) by mid-unit so
    exp(u+1) never WAR-stalls on the pT pool).

The reference reshapes K as (T, DH, H) (head axis interleaved), unlike
Q/V (H, DH) — handled by a host-side column permutation of Wk/bk.
"""

import sys

if "/opt/trn_rl_repo" not in sys.path:
    sys.path.insert(0, "/opt/trn_rl_repo")

import numpy as np
import ml_dtypes

import concourse.bass as bass
import concourse.bacc as bacc
import concourse.mybir as mybir
import concourse.tile as tile
from concourse import bass_utils

BF16 = mybir.dt.bfloat16
F32 = mybir.dt.float32
bf16 = ml_dtypes.bfloat16

B, F, T, C, H, DH = 4, 2048, 2048, 1024, 16, 64
HL = 8          # heads per core
COLS = HL * DH  # 512 projected columns per core
ALPHA = 0.125   # 1/sqrt(64)
NCORES = 8
KT = C // 128   # 8 contraction tiles for projections
NFT = F // 128  # 16 F tiles
NTT = T // 128  # 16 T tiles
NPAIR = 4       # head pairs per core

# Toggled by test.py for profiling runs.
PROFILE = False
LAST_RESULTS = None

_nc_cache = None


def _emit(tc, nc, aps):
    xT, yT, maskT, wq, wk, wv, wq0d, wk0d, bq, bk, out = aps
    Exp = mybir.ActivationFunctionType.Exp

    import contextlib

    with contextlib.ExitStack() as ctx:
        pool = ctx.enter_context(tc.tile_pool(name="static", bufs=1))
        xTp = ctx.enter_context(tc.tile_pool(name="xTp", bufs=1))
        qTp = ctx.enter_context(tc.tile_pool(name="qTp", bufs=2))
        maskp = ctx.enter_context(tc.tile_pool(name="maskp", bufs=3))
        pTp = ctx.enter_context(tc.tile_pool(name="pTp", bufs=2))
        outp = ctx.enter_context(tc.tile_pool(name="outp", bufs=1))
        dinvp = ctx.enter_context(tc.tile_pool(name="dinvp", bufs=4))
        psum_s = ctx.enter_context(tc.tile_pool(name="psum_s", bufs=3, space="PSUM"))
        psum_ctx = ctx.enter_context(tc.tile_pool(name="psum_ctx", bufs=2, space="PSUM"))

        # Static tiles — yT and kT split per 512-col chunk for fine deps.
        kT4 = [[pool.tile([128, 512], BF16, name=f"kT{cb}_{c}", tag=f"kT{cb}_{c}")
                for c in range(4)] for cb in range(4)]
        yT4 = [[pool.tile([128, 512], BF16, name=f"yT{k}_{c}", tag=f"yT{k}_{c}")
                for c in range(4)] for k in range(KT)]
        v = [pool.tile([128, HL * 65], BF16, name=f"v{tt}", tag=f"v{tt}")
             for tt in range(NTT)]
        wq_sb = [pool.tile([128, COLS], BF16, name=f"wq{k}", tag=f"wq{k}") for k in range(KT)]
        wk_sb = [pool.tile([128, COLS], BF16, name=f"wk{k}", tag=f"wk{k}") for k in range(KT)]
        wq0 = [pool.tile([128, 128], BF16, name=f"wq0_{k}", tag=f"wq0_{k}") for k in range(KT)]
        wk0 = [pool.tile([128, 128], BF16, name=f"wk0_{k}", tag=f"wk0_{k}") for k in range(KT)]
        wv_sb = [pool.tile([128, COLS], BF16, name=f"wv{k}", tag=f"wv{k}") for k in range(KT)]
        bq_sb = pool.tile([128, 4], F32, name="bq_sb", tag="bq_sb")
        bk_sb = pool.tile([128, 4], F32, name="bk_sb", tag="bk_sb")

        xT_r = xT.rearrange("c (k p) f -> c p k f", p=128)
        maskT_r = maskT.rearrange("c (tt p) f -> c p tt f", p=128)
        out_r = out.rearrange("(g p) c -> p g c", p=128)

        # ---- upfront DMA queue (sync engine FIFO). Order: everything the
        # first exp needs (y chunk0, wk, wq, x chunk0) lands by ~12us.
        nc.sync.dma_start(bk_sb[:], bk[:])
        nc.sync.dma_start(bq_sb[:], bq[:])
        warm_sb = pool.tile([1, 8], F32, name="warm_sb", tag="warm_sb")
        nc.vector.memset(warm_sb[:], 0.0)
        nc.scalar.activation(warm_sb[:], warm_sb[:], Exp)

        # 4 parallel DMA queues (~190 GB/s each): sync=y0/y1, vector=wv/y2/y3,
        # scalar=fc0 masks, gpsimd=weights+x0. First-exp prereqs land ~10us.
        for k in range(KT):
            nc.gpsimd.dma_start(wk0[k][:], wk0d[k * 128:(k + 1) * 128, :])
        for k in range(KT):
            nc.gpsimd.dma_start(wq0[k][:], wq0d[k * 128:(k + 1) * 128, :])
        for k in range(KT):
            nc.sync.dma_start(yT4[k][0][:], yT[0, k * 128:(k + 1) * 128, :])
        for k in range(KT):
            nc.scalar.dma_start(wv_sb[k][:], wv[k * 128:(k + 1) * 128, :])
        mask_h = {}
        for half in range(2):
            mh = maskp.tile([128, 8, 512], BF16, name="mh", tag="mask")
            nc.scalar.dma_start(mh[:], maskT_r[0, :, half * 8:(half + 1) * 8, :])
            mask_h[(0, half)] = mh
        xTt = xTp.tile([128, KT, 512], BF16, name="xTt", tag="xT")
        nc.gpsimd.dma_start(xTt[:], xT_r[0])
        for k in range(KT):
            nc.sync.dma_start(yT4[k][1][:], yT[1, k * 128:(k + 1) * 128, :])
        for k in range(KT):
            nc.sync.dma_start(yT4[k][2][:], yT[2, k * 128:(k + 1) * 128, :])
        for k in range(KT):
            nc.gpsimd.dma_start(wq_sb[k][:], wq[k * 128:(k + 1) * 128, :])
        for k in range(KT):
            nc.sync.dma_start(yT4[k][3][:], yT[3, k * 128:(k + 1) * 128, :])
        for k in range(KT):
            nc.gpsimd.dma_start(wk_sb[k][:], wk[k * 128:(k + 1) * 128, :])

        # ---- chain emitters (PE work units) ----
        def k_chain(cb, tcc):
            ps = psum_s.tile([128, 1024], F32, name="ps_s", tag="s")[:, 0:512]
            for k in range(KT):
                nc.tensor.matmul(
                    ps[:],
                    wk0[k][:] if cb == 0 else wk_sb[k][:, cb * 128:(cb + 1) * 128],
                    yT4[k][tcc][:],
                    start=(k == 0),
                    stop=(k == KT - 1),
                )
            nc.vector.tensor_scalar_add(kT4[cb][tcc][:], ps[:], bk_sb[:, cb:cb + 1])

        def v_chain(tt):
            ps = psum_s.tile([128, 1024], F32, name="ps_s", tag="s")[:, 0:512]
            for k in range(KT):
                nc.tensor.matmul(
                    ps[:],
                    yT4[k][tt // 4][:, (tt % 4) * 128:(tt % 4 + 1) * 128],
                    wv_sb[k][:],
                    start=(k == 0),
                    stop=(k == KT - 1),
                )
            vview = v[tt].rearrange("p (h c) -> p h c", c=65)
            nc.vector.tensor_copy(
                vview[:, :, 0:64], ps.rearrange("p (h c) -> p h c", c=64)[:]
            )
            nc.vector.memset(vview[:, :, 64:65], 1.0)

        qT_tiles = {}

        def q_chain(fc, cb, xt):
            qt = qT_tiles[fc]
            ps = psum_s.tile([128, 1024], F32, name="ps_s", tag="s")[:, 0:512]
            for k in range(KT):
                nc.tensor.matmul(
                    ps[:],
                    wq0[k][:] if cb == 0 else wq_sb[k][:, cb * 128:(cb + 1) * 128],
                    xt[:, k, :],
                    start=(k == 0),
                    stop=(k == KT - 1),
                )
            nc.vector.tensor_scalar_add(qt[:, cb, :], ps[:], bq_sb[:, cb:cb + 1])

        # ---- unit machinery ----
        pT_store = {}
        ctx_ps = {}
        out_tiles = {}

        def emit_scores_tt(u, tt):
            fc, pair = u // 4, u % 4
            qt = qT_tiles[fc]
            ps = psum_s.tile([128, 1024], F32, name="ps_s", tag="s")
            for hh in range(2):
                nc.tensor.matmul(
                    ps[:, hh * 512:(hh + 1) * 512],
                    kT4[pair][tt // 4][hh * 64:(hh + 1) * 64,
                                       (tt % 4) * 128:(tt % 4 + 1) * 128],
                    qt[hh * 64:(hh + 1) * 64, pair, :],
                    start=True, stop=True,
                )
            nc.scalar.activation(pT_store[u][:, tt, :], ps[:], Exp, scale=ALPHA)

        def emit_mask_4tt(u, tt0):
            fc = u // 4
            mh = mask_h[(fc, tt0 // 8)]
            o = pT_store[u][:, tt0:tt0 + 4, :].rearrange(
                "p t (h c) -> p t h c", c=512
            )
            m = mh[:, tt0 % 8: tt0 % 8 + 4, :].unsqueeze(2).broadcast_to(
                [128, 4, 2, 512]
            )
            nc.vector.tensor_mul(o[:], o[:], m)

        def emit_ctx_chain(cu, j, half=None):
            """Chain j in 0..7: (hh = j//4, ft = j%4); half=0/1 emits tt 0-7 /
            8-15. After each head's last chain, evacuate it."""
            pair = cu % 4
            hh, ft = j // 4, j % 4
            if ft == 0 and half in (None, 0):
                ctx_ps.setdefault(cu, {})[hh] = psum_ctx.tile(
                    [128, 512], F32, name="pc", tag="pc"
                )
            pc = ctx_ps[cu][hh]
            pt = pT_store[cu]
            h = pair * 2 + hh
            tts = range(NTT) if half is None else range(half * 8, half * 8 + 8)
            for tt in tts:
                nc.tensor.matmul(
                    pc[:, ft * 65:ft * 65 + 65],
                    pt[:, tt, hh * 512 + ft * 128: hh * 512 + (ft + 1) * 128],
                    v[tt][:, h * 65:(h + 1) * 65],
                    start=(tt == 0),
                    stop=(tt == NTT - 1),
                )
            if ft == 3 and half in (None, 1):
                emit_ctx_evac_h(cu, hh)

        def emit_ctx_evac_h(cu, hh):
            fc, pair = cu // 4, cu % 4
            if cu not in out_tiles:
                out_tiles[cu] = outp.tile([128, 4, 128], F32, name="outt", tag="out")
            ot = out_tiles[cu]
            pc = ctx_ps[cu][hh]
            dinv = dinvp.tile([128, 4], F32, name="dinv", tag="dinv")
            nc.vector.reciprocal(
                dinv.rearrange("p (a b) -> p a b", b=1)[:],
                pc[:, 0:260].rearrange("p (ft c) -> p ft c", c=65)[:, :, 64:65],
            )
            nc.vector.tensor_mul(
                ot[:, :, hh * 64:(hh + 1) * 64],
                pc[:, 0:260].rearrange("p (ft c) -> p ft c", c=65)[:, :, 0:64],
                dinv.unsqueeze(2).broadcast_to([128, 4, 64]),
            )
            if hh == 1:
                ctx_ps.pop(cu)
                del pT_store[cu]
                nc.gpsimd.dma_start(
                    out_r[:, fc * 4:(fc + 1) * 4, pair * 128:(pair + 1) * 128],
                    ot[:],
                )
                del out_tiles[cu]

        def unit(u, fillers, ctx_u, dmas=(), late_dmas=(), ctx_late=False,
                 late_masks=False):
            for d in dmas:
                d()
            pT_store[u] = pTp.tile([128, NTT, 1024], BF16, name="pT", tag="pT")
            nf = len(fillers)
            fspan = 8 if ctx_late else NTT
            fi = 0
            for tt in range(NTT):
                emit_scores_tt(u, tt)
                if late_masks:
                    # all 4 mask ops at slots 12-15: keeps early DVE FIFO
                    # free of mask-DMA waits during the startup units
                    if tt >= 12:
                        emit_mask_4tt(u, 4 * (tt - 12))
                elif tt % 4 == 3:
                    emit_mask_4tt(u, tt - 3)
                want = nf if tt >= fspan else (tt + 1) * nf // fspan
                while fi < want:
                    fillers[fi]()
                    fi += 1
                if ctx_u is not None:
                    if ctx_late and tt >= 8:
                        emit_ctx_chain(ctx_u, tt - 8)
                    elif not ctx_late:
                        emit_ctx_chain(ctx_u, tt // 2, half=tt % 2)
                if tt == 10:
                    for d in late_dmas:
                        d()

        # deferred DMA emitters
        def dma_xt(fc):
            def go():
                xt = xTp.tile([128, KT, 512], BF16, name="xTt", tag="xT")
                nc.sync.dma_start(xt[:], xT_r[fc])
                dma_xt.tiles[fc] = xt
            return go
        dma_xt.tiles = {0: xTt}

        def dma_mask(fc, half):
            def go():
                mh = maskp.tile([128, 8, 512], BF16, name="mh", tag="mask")
                nc.sync.dma_start(
                    mh[:], maskT_r[fc, :, half * 8:(half + 1) * 8, :]
                )
                mask_h[(fc, half)] = mh
            return go

        # ---- prologue ----
        qT_tiles[0] = qTp.tile([128, 4, 512], BF16, name="qTt", tag="qT")
        k_chain(0, 0)
        q_chain(0, 0, xTt)

        def q_fillers(fc):
            qT_tiles[fc] = qTp.tile([128, 4, 512], BF16, name="qTt", tag="qT")
            return [
                (lambda cb=cb: q_chain(fc, cb, dma_xt.tiles[fc])) for cb in range(4)
            ]

        # ---- 16 units ----
        unit(0, [lambda: k_chain(0, 1), lambda: k_chain(0, 2),
                 lambda: q_chain(0, 1, xTt), lambda: k_chain(1, 0),
                 lambda: k_chain(1, 1), lambda: k_chain(1, 2),
                 lambda: k_chain(0, 3), lambda: k_chain(1, 3)]
                + [lambda t=t: v_chain(t) for t in range(4)],
             None, dmas=(dma_xt(1),), late_masks=True)
        unit(1, [lambda t=t: v_chain(t) for t in range(4, 16)]
                + [lambda t=t: k_chain(2, t) for t in range(4)]
                + [lambda: q_chain(0, 2, xTt)],
             0, ctx_late=True, late_masks=True)
        unit(2, [lambda t=t: k_chain(3, t) for t in range(4)]
                + [lambda: q_chain(0, 3, xTt)], 1)
        unit(3, q_fillers(1), 2, late_dmas=(dma_mask(1, 0), dma_mask(1, 1)))
        unit(4, [], 3)
        unit(5, [], 4, dmas=(dma_xt(2),))
        unit(6, [], 5)
        unit(7, q_fillers(2), 6, late_dmas=(dma_mask(2, 0), dma_mask(2, 1)))
        unit(8, [], 7)
        unit(9, [], 8, dmas=(dma_xt(3),))
        unit(10, [], 9)
        unit(11, q_fillers(3), 10, late_dmas=(dma_mask(3, 0), dma_mask(3, 1)))
        unit(12, [], 11)
        unit(13, [], 12)
        unit(14, [], 13)
        unit(15, [], 14)
        # tail: context of the last unit
        for j in range(8):
            emit_ctx_chain(15, j)


def _build():
    global _nc_cache
    if _nc_cache is not None:
        return _nc_cache
    nc = bacc.Bacc(
        "TRN2",
        target_bir_lowering=False,
        debug=False,
        enable_asserts=False,
        num_devices=NCORES,
    )
    xT = nc.dram_tensor("xT", [4, C, 512], BF16, kind="ExternalInput").ap()
    yT = nc.dram_tensor("yT", [4, C, 512], BF16, kind="ExternalInput").ap()
    maskT = nc.dram_tensor("maskT", [4, T, 512], BF16, kind="ExternalInput").ap()
    wq = nc.dram_tensor("wq", [C, COLS], BF16, kind="ExternalInput").ap()
    wk = nc.dram_tensor("wk", [C, COLS], BF16, kind="ExternalInput").ap()
    wq0d = nc.dram_tensor("wq0", [C, 128], BF16, kind="ExternalInput").ap()
    wk0d = nc.dram_tensor("wk0", [C, 128], BF16, kind="ExternalInput").ap()
    wv = nc.dram_tensor("wv", [C, COLS], BF16, kind="ExternalInput").ap()
    bq = nc.dram_tensor("bq", [128, 4], F32, kind="ExternalInput").ap()
    bk = nc.dram_tensor("bk", [128, 4], F32, kind="ExternalInput").ap()
    out = nc.dram_tensor("out", [F, COLS], F32, kind="ExternalOutput").ap()

    with tile.TileContext(nc) as tc:
        _emit(tc, nc, (xT, yT, maskT, wq, wk, wv, wq0d, wk0d, bq, bk, out))
    nc.compile()
    _nc_cache = nc
    return nc


def _kperm(hg):
    """Local K column (pair*128 + hh*64 + d) -> global Wk column d*H + h_g."""
    idx = np.empty(COLS, dtype=np.int64)
    for pair in range(NPAIR):
        for hh in range(2):
            h_g = hg * HL + pair * 2 + hh
            for d in range(DH):
                idx[pair * 128 + hh * 64 + d] = d * H + h_g
    return idx


def make_in_maps(from_tensor, to_tensor, mask, Wq, bq, Wk, bk, Wv, bv):
    per_b = {}
    for b in range(B):
        xTb = from_tensor[b].T.astype(bf16)     # [C, F]
        yTb = to_tensor[b].T.astype(bf16)       # [C, T]
        mTb = mask[b].T.astype(bf16)            # [T, F]
        per_b[b] = (
            np.ascontiguousarray(xTb.reshape(C, 4, 512).transpose(1, 0, 2)),
            np.ascontiguousarray(yTb.reshape(C, 4, 512).transpose(1, 0, 2)),
            np.ascontiguousarray(mTb.reshape(T, 4, 512).transpose(1, 0, 2)),
        )
    in_maps = []
    for i in range(NCORES):
        b, hg = i // 2, i % 2
        xTb, yTb, mTb = per_b[b]
        sl = slice(hg * COLS, (hg + 1) * COLS)
        kidx = _kperm(hg)
        in_maps.append(
            {
                "xT": xTb,
                "yT": yTb,
                "maskT": mTb,
                "wq": np.ascontiguousarray(Wq[:, sl]).astype(bf16),
                "wk": np.ascontiguousarray(Wk[:, kidx]).astype(bf16),
                "wq0": np.ascontiguousarray(Wq[:, sl][:, 0:128]).astype(bf16),
                "wk0": np.ascontiguousarray(Wk[:, kidx][:, 0:128]).astype(bf16),
                "wv": np.ascontiguousarray(Wv[:, sl]).astype(bf16),
                "bq": np.ascontiguousarray(
                    bq[sl].astype(np.float32).reshape(4, 128).T
                ),
                "bk": np.ascontiguousarray(
                    bk[kidx].astype(np.float32).reshape(4, 128).T
                ),
            }
        )
    return in_maps


def kernel(from_tensor, to_tensor, mask, Wq, bq, Wk, bk, Wv, bv):
    global LAST_RESULTS
    from_tensor = np.asarray(from_tensor, dtype=np.float32)
    to_tensor = np.asarray(to_tensor, dtype=np.float32)
    mask_np = np.asarray(mask)
    Wq = np.asarray(Wq, dtype=np.float32)
    Wk = np.asarray(Wk, dtype=np.float32)
    Wv = np.asarray(Wv, dtype=np.float32)
    bq = np.asarray(bq, dtype=np.float32)
    bk = np.asarray(bk, dtype=np.float32)
    bv = np.asarray(bv, dtype=np.float32)

    nc = _build()
    in_maps = make_in_maps(
        from_tensor, to_tensor, mask_np, Wq, bq, Wk, bk, Wv, bv
    )
    res = bass_utils.run_bass_kernel_spmd(
        nc, in_maps, core_ids=list(range(NCORES)), trace=PROFILE
    )
    LAST_RESULTS = res
    full = np.empty((B, F, H * DH), np.float32)
    for i in range(NCORES):
        b, hg = i // 2, i % 2
        # context bias is additive: ctx(v + bv) = ctx(v) + bv
        full[b, :, hg * COLS:(hg + 1) * COLS] = (
            res.results[i]["out"] + bv[hg * COLS:(hg + 1) * COLS]
        )
    return full


# revision 23
# speedup vs baseline: 1.0121x; 1.0121x over previous
"""Trainium2 Bass kernel for nn_Attention_12137577578573 (v4).

Full multi-head attention (QKV projection + masked softmax + context) for
B=4, F=T=2048, CF=CT=1024, H=16, DH=64, sharded over 8 NeuronCores as
(batch b, head-group hg): core i = (b = i // 2, hg = i % 2), each core
computing 1 batch x 8 heads.

Layout strategy (contraction dim on partitions):
  - host pre-transposes from/to tensors -> xT/yT [C, F]; yT and K^T live
    as per-512-column-chunk tiles so early chains don't dep-wait on whole
    tensors (Tile deps are tile-granular).
  - Q^T, K^T in transposed layout [cols, F]/[cols, T]; 2 heads per
    128-partition tile.
  - scores S^T [T, F] (T on partitions); softmax denominator via a
    ones-column appended to V; bv folded out on the host (context bias
    is additive); mask applied as P = exp(alpha*S) * maskT.
  - exp emission per tt ([128,1024] PSUM -> bf16 SBUF); mask multiplies
    are emitted one unit LATE (during unit u+1) so their DMA-wait can't
    head-of-line-block the DVE FIFO behind projection bias-adds.
  - context of unit u-1 runs tt-outer, 2 t-tiles per slot in slots 1-8 of
    unit u (consumes v[tt] incrementally, frees pT(u-1) by mid-unit so
    exp(u+1) never WAR-stalls on the pT pool).

The reference reshapes K as (T, DH, H) (head axis interleaved), unlike
Q/V (H, DH) — handled by a host-side column permutation of Wk/bk.
"""

import sys

if "/opt/trn_rl_repo" not in sys.path:
    sys.path.insert(0, "/opt/trn_rl_repo")

import numpy as np
import ml_dtypes

import concourse.bass as bass
import concourse.bacc as bacc
import concourse.mybir as mybir
import concourse.tile as tile
from concourse import bass_utils

BF16 = mybir.dt.bfloat16
F32 = mybir.dt.float32
bf16 = ml_dtypes.bfloat16

B, F, T, C, H, DH = 4, 2048, 2048, 1024, 16, 64
HL = 8          # heads per core
COLS = HL * DH  # 512 projected columns per core
ALPHA = 0.125   # 1/sqrt(64)
NCORES = 8
KT = C // 128   # 8 contraction tiles for projections
NFT = F // 128  # 16 F tiles
NTT = T // 128  # 16 T tiles
NPAIR = 4       # head pairs per core

# Toggled by test.py for profiling runs.
PROFILE = False
LAST_RESULTS = None

_nc_cache = None


def _emit(tc, nc, aps):
    xT, yT, maskT, wq, wk, wv, wq0d, wk0d, bq, bk, out = aps
    Exp = mybir.ActivationFunctionType.Exp

    import contextlib

    with contextlib.ExitStack() as ctx:
        pool = ctx.enter_context(tc.tile_pool(name="static", bufs=1))
        xTp = ctx.enter_context(tc.tile_pool(name="xTp", bufs=1))
        qTp = ctx.enter_context(tc.tile_pool(name="qTp", bufs=2))
        maskp = ctx.enter_context(tc.tile_pool(name="maskp", bufs=3))
        pTp = ctx.enter_context(tc.tile_pool(name="pTp", bufs=2))
        outp = ctx.enter_context(tc.tile_pool(name="outp", bufs=1))
        dinvp = ctx.enter_context(tc.tile_pool(name="dinvp", bufs=4))
        psum_s = ctx.enter_context(tc.tile_pool(name="psum_s", bufs=3, space="PSUM"))
        psum_ctx = ctx.enter_context(tc.tile_pool(name="psum_ctx", bufs=2, space="PSUM"))

        # Static tiles — yT and kT split per 512-col chunk for fine deps.
        kT4 = [[pool.tile([128, 512], BF16, name=f"kT{cb}_{c}", tag=f"kT{cb}_{c}")
                for c in range(4)] for cb in range(4)]
        yT4 = [[pool.tile([128, 512], BF16, name=f"yT{k}_{c}", tag=f"yT{k}_{c}")
                for c in range(4)] for k in range(KT)]
        v = [pool.tile([128, HL * 65], BF16, name=f"v{tt}", tag=f"v{tt}")
             for tt in range(NTT)]
        wq_sb = [pool.tile([128, COLS], BF16, name=f"wq{k}", tag=f"wq{k}") for k in range(KT)]
        wk_sb = [pool.tile([128, COLS], BF16, name=f"wk{k}", tag=f"wk{k}") for k in range(KT)]
        wq0 = [pool.tile([128, 128], BF16, name=f"wq0_{k}", tag=f"wq0_{k}") for k in range(KT)]
        wk0 = [pool.tile([128, 128], BF16, name=f"wk0_{k}", tag=f"wk0_{k}") for k in range(KT)]
        wv_sb = [pool.tile([128, COLS], BF16, name=f"wv{k}", tag=f"wv{k}") for k in range(KT)]
        bq_sb = pool.tile([128, 4], F32, name="bq_sb", tag="bq_sb")
        bk_sb = pool.tile([128, 4], F32, name="bk_sb", tag="bk_sb")

        xT_r = xT.rearrange("c (k p) f -> c p k f", p=128)
        maskT_r = maskT.rearrange("c (tt p) f -> c p tt f", p=128)
        out_r = out.rearrange("(g p) c -> p g c", p=128)

        # ---- upfront DMA queue (sync engine FIFO). Order: everything the
        # first exp needs (y chunk0, wk, wq, x chunk0) lands by ~12us.
        nc.sync.dma_start(bk_sb[:], bk[:])
        nc.sync.dma_start(bq_sb[:], bq[:])
        warm_sb = pool.tile([1, 8], F32, name="warm_sb", tag="warm_sb")
        nc.vector.memset(warm_sb[:], 0.0)
        nc.scalar.activation(warm_sb[:], warm_sb[:], Exp)

        # 4 parallel DMA queues (~190 GB/s each): sync=y0/y1, vector=wv/y2/y3,
        # scalar=fc0 masks, gpsimd=weights+x0. First-exp prereqs land ~10us.
        for k in range(KT):
            nc.gpsimd.dma_start(wk0[k][:], wk0d[k * 128:(k + 1) * 128, :])
        for k in range(KT):
            nc.gpsimd.dma_start(wq0[k][:], wq0d[k * 128:(k + 1) * 128, :])
        for k in range(KT):
            nc.sync.dma_start(yT4[k][0][:], yT[0, k * 128:(k + 1) * 128, :])
        xTt = xTp.tile([128, KT, 512], BF16, name="xTt", tag="xT")
        nc.gpsimd.dma_start(xTt[:], xT_r[0])
        for k in range(KT):
            nc.sync.dma_start(yT4[k][1][:], yT[1, k * 128:(k + 1) * 128, :])
        for k in range(KT):
            nc.gpsimd.dma_start(wq_sb[k][:], wq[k * 128:(k + 1) * 128, :])
        for k in range(KT):
            nc.sync.dma_start(yT4[k][2][:], yT[2, k * 128:(k + 1) * 128, :])
        for k in range(KT):
            nc.gpsimd.dma_start(wk_sb[k][:], wk[k * 128:(k + 1) * 128, :])
        for k in range(KT):
            nc.sync.dma_start(yT4[k][3][:], yT[3, k * 128:(k + 1) * 128, :])
        for k in range(KT):
            nc.gpsimd.dma_start(wv_sb[k][:], wv[k * 128:(k + 1) * 128, :])
        mask_h = {}
        for half in range(2):
            mh = maskp.tile([128, 8, 512], BF16, name="mh", tag="mask")
            nc.sync.dma_start(mh[:], maskT_r[0, :, half * 8:(half + 1) * 8, :])
            mask_h[(0, half)] = mh

        # ---- chain emitters (PE work units) ----
        def k_chain(cb, tcc):
            ps = psum_s.tile([128, 1024], F32, name="ps_s", tag="s")[:, 0:512]
            for k in range(KT):
                nc.tensor.matmul(
                    ps[:],
                    wk0[k][:] if cb == 0 else wk_sb[k][:, cb * 128:(cb + 1) * 128],
                    yT4[k][tcc][:],
                    start=(k == 0),
                    stop=(k == KT - 1),
                )
            nc.vector.tensor_scalar_add(kT4[cb][tcc][:], ps[:], bk_sb[:, cb:cb + 1])

        def v_chain(tt):
            ps = psum_s.tile([128, 1024], F32, name="ps_s", tag="s")[:, 0:512]
            for k in range(KT):
                nc.tensor.matmul(
                    ps[:],
                    yT4[k][tt // 4][:, (tt % 4) * 128:(tt % 4 + 1) * 128],
                    wv_sb[k][:],
                    start=(k == 0),
                    stop=(k == KT - 1),
                )
            vview = v[tt].rearrange("p (h c) -> p h c", c=65)
            nc.vector.tensor_copy(
                vview[:, :, 0:64], ps.rearrange("p (h c) -> p h c", c=64)[:]
            )
            nc.vector.memset(vview[:, :, 64:65], 1.0)

        qT_tiles = {}

        def q_chain(fc, cb, xt):
            qt = qT_tiles[fc]
            ps = psum_s.tile([128, 1024], F32, name="ps_s", tag="s")[:, 0:512]
            for k in range(KT):
                nc.tensor.matmul(
                    ps[:],
                    wq0[k][:] if cb == 0 else wq_sb[k][:, cb * 128:(cb + 1) * 128],
                    xt[:, k, :],
                    start=(k == 0),
                    stop=(k == KT - 1),
                )
            nc.vector.tensor_scalar_add(qt[:, cb, :], ps[:], bq_sb[:, cb:cb + 1])

        # ---- unit machinery ----
        pT_store = {}
        ctx_ps = {}
        out_tiles = {}

        def emit_scores_tt(u, tt):
            fc, pair = u // 4, u % 4
            qt = qT_tiles[fc]
            ps = psum_s.tile([128, 1024], F32, name="ps_s", tag="s")
            for hh in range(2):
                nc.tensor.matmul(
                    ps[:, hh * 512:(hh + 1) * 512],
                    kT4[pair][tt // 4][hh * 64:(hh + 1) * 64,
                                       (tt % 4) * 128:(tt % 4 + 1) * 128],
                    qt[hh * 64:(hh + 1) * 64, pair, :],
                    start=True, stop=True,
                )
            nc.scalar.activation(pT_store[u][:, tt, :], ps[:], Exp, scale=ALPHA)

        def emit_mask_4tt(u, tt0):
            fc = u // 4
            mh = mask_h[(fc, tt0 // 8)]
            o = pT_store[u][:, tt0:tt0 + 4, :].rearrange(
                "p t (h c) -> p t h c", c=512
            )
            m = mh[:, tt0 % 8: tt0 % 8 + 4, :].unsqueeze(2).broadcast_to(
                [128, 4, 2, 512]
            )
            nc.vector.tensor_mul(o[:], o[:], m)

        def emit_ctx_chain(cu, j, half=None):
            """Chain j in 0..7: (hh = j//4, ft = j%4); half=0/1 emits tt 0-7 /
            8-15. After each head's last chain, evacuate it."""
            pair = cu % 4
            hh, ft = j // 4, j % 4
            if ft == 0 and half in (None, 0):
                ctx_ps.setdefault(cu, {})[hh] = psum_ctx.tile(
                    [128, 512], F32, name="pc", tag="pc"
                )
            pc = ctx_ps[cu][hh]
            pt = pT_store[cu]
            h = pair * 2 + hh
            tts = range(NTT) if half is None else range(half * 8, half * 8 + 8)
            for tt in tts:
                nc.tensor.matmul(
                    pc[:, ft * 65:ft * 65 + 65],
                    pt[:, tt, hh * 512 + ft * 128: hh * 512 + (ft + 1) * 128],
                    v[tt][:, h * 65:(h + 1) * 65],
                    start=(tt == 0),
                    stop=(tt == NTT - 1),
                )
            if ft == 3 and half in (None, 1):
                emit_ctx_evac_h(cu, hh)

        def emit_ctx_evac_h(cu, hh):
            fc, pair = cu // 4, cu % 4
            if cu not in out_tiles:
                out_tiles[cu] = outp.tile([128, 4, 128], F32, name="outt", tag="out")
            ot = out_tiles[cu]
            pc = ctx_ps[cu][hh]
            dinv = dinvp.tile([128, 4], F32, name="dinv", tag="dinv")
            nc.vector.reciprocal(
                dinv.rearrange("p (a b) -> p a b", b=1)[:],
                pc[:, 0:260].rearrange("p (ft c) -> p ft c", c=65)[:, :, 64:65],
            )
            nc.vector.tensor_mul(
                ot[:, :, hh * 64:(hh + 1) * 64],
                pc[:, 0:260].rearrange("p (ft c) -> p ft c", c=65)[:, :, 0:64],
                dinv.unsqueeze(2).broadcast_to([128, 4, 64]),
            )
            if hh == 1:
                ctx_ps.pop(cu)
                del pT_store[cu]
                nc.gpsimd.dma_start(
                    out_r[:, fc * 4:(fc + 1) * 4, pair * 128:(pair + 1) * 128],
                    ot[:],
                )
                del out_tiles[cu]

        def unit(u, fillers, ctx_u, dmas=(), late_dmas=(), ctx_late=False,
                 late_masks=False):
            for d in dmas:
                d()
            pT_store[u] = pTp.tile([128, NTT, 1024], BF16, name="pT", tag="pT")
            nf = len(fillers)
            fspan = 8 if ctx_late else NTT
            fi = 0
            for tt in range(NTT):
                emit_scores_tt(u, tt)
                if late_masks:
                    # all 4 mask ops at slots 12-15: keeps early DVE FIFO
                    # free of mask-DMA waits during the startup units
                    if tt >= 12:
                        emit_mask_4tt(u, 4 * (tt - 12))
                elif tt % 4 == 3:
                    emit_mask_4tt(u, tt - 3)
                want = nf if tt >= fspan else (tt + 1) * nf // fspan
                while fi < want:
                    fillers[fi]()
                    fi += 1
                if ctx_u is not None:
                    if ctx_late and tt >= 8:
                        emit_ctx_chain(ctx_u, tt - 8)
                    elif not ctx_late:
                        emit_ctx_chain(ctx_u, tt // 2, half=tt % 2)
                if tt == 10:
                    for d in late_dmas:
                        d()

        # deferred DMA emitters
        def dma_xt(fc):
            def go():
                xt = xTp.tile([128, KT, 512], BF16, name="xTt", tag="xT")
                nc.sync.dma_start(xt[:], xT_r[fc])
                dma_xt.tiles[fc] = xt
            return go
        dma_xt.tiles = {0: xTt}

        def dma_mask(fc, half):
            def go():
                mh = maskp.tile([128, 8, 512], BF16, name="mh", tag="mask")
                nc.sync.dma_start(
                    mh[:], maskT_r[fc, :, half * 8:(half + 1) * 8, :]
                )
                mask_h[(fc, half)] = mh
            return go

        # ---- prologue ----
        qT_tiles[0] = qTp.tile([128, 4, 512], BF16, name="qTt", tag="qT")
        k_chain(0, 0)
        q_chain(0, 0, xTt)

        def q_fillers(fc):
            qT_tiles[fc] = qTp.tile([128, 4, 512], BF16, name="qTt", tag="qT")
            return [
                (lambda cb=cb: q_chain(fc, cb, dma_xt.tiles[fc])) for cb in range(4)
            ]

        # ---- 16 units ----
        unit(0, [lambda: k_chain(0, 1), lambda: k_chain(0, 2),
                 lambda: q_chain(0, 1, xTt), lambda: k_chain(1, 0),
                 lambda: k_chain(1, 1), lambda: k_chain(1, 2),
                 lambda: k_chain(0, 3), lambda: k_chain(1, 3)]
                + [lambda t=t: v_chain(t) for t in range(4)],
             None, dmas=(dma_xt(1),), late_masks=True)
        unit(1, [lambda t=t: v_chain(t) for t in range(4, 16)]
                + [lambda t=t: k_chain(2, t) for t in range(4)]
                + [lambda: q_chain(0, 2, xTt)],
             0, ctx_late=True, late_masks=True)
        unit(2, [lambda t=t: k_chain(3, t) for t in range(4)]
                + [lambda: q_chain(0, 3, xTt)], 1)
        unit(3, q_fillers(1), 2, late_dmas=(dma_mask(1, 0), dma_mask(1, 1)))
        unit(4, [], 3)
        unit(5, [], 4, dmas=(dma_xt(2),))
        unit(6, [], 5)
        unit(7, q_fillers(2), 6, late_dmas=(dma_mask(2, 0), dma_mask(2, 1)))
        unit(8, [], 7)
        unit(9, [], 8, dmas=(dma_xt(3),))
        unit(10, [], 9)
        unit(11, q_fillers(3), 10, late_dmas=(dma_mask(3, 0), dma_mask(3, 1)))
        unit(12, [], 11)
        unit(13, [], 12)
        unit(14, [], 13)
        unit(15, [], 14)
        # tail: context of the last unit
        for j in range(8):
            emit_ctx_chain(15, j)


def _build():
    global _nc_cache
    if _nc_cache is not None:
        return _nc_cache
    nc = bacc.Bacc(
        "TRN2",
        target_bir_lowering=False,
        debug=False,
        enable_asserts=False,
        num_devices=NCORES,
    )
    xT = nc.dram_tensor("xT", [4, C, 512], BF16, kind="ExternalInput").ap()
    yT = nc.dram_tensor("yT", [4, C, 512], BF16, kind="ExternalInput").ap()
    maskT = nc.dram_tensor("maskT", [4, T, 512], BF16, kind="ExternalInput").ap()
    wq = nc.dram_tensor("wq", [C, COLS], BF16, kind="ExternalInput").ap()
    wk = nc.dram_tensor("wk", [C, COLS], BF16, kind="ExternalInput").ap()
    wq0d = nc.dram_tensor("wq0", [C, 128], BF16, kind="ExternalInput").ap()
    wk0d = nc.dram_tensor("wk0", [C, 128], BF16, kind="ExternalInput").ap()
    wv = nc.dram_tensor("wv", [C, COLS], BF16, kind="ExternalInput").ap()
    bq = nc.dram_tensor("bq", [128, 4], F32, kind="ExternalInput").ap()
    bk = nc.dram_tensor("bk", [128, 4], F32, kind="ExternalInput").ap()
    out = nc.dram_tensor("out", [F, COLS], F32, kind="ExternalOutput").ap()

    with tile.TileContext(nc) as tc:
        _emit(tc, nc, (xT, yT, maskT, wq, wk, wv, wq0d, wk0d, bq, bk, out))
    nc.compile()
    _nc_cache = nc
    return nc


def _kperm(hg):
    """Local K column (pair*128 + hh*64 + d) -> global Wk column d*H + h_g."""
    idx = np.empty(COLS, dtype=np.int64)
    for pair in range(NPAIR):
        for hh in range(2):
            h_g = hg * HL + pair * 2 + hh
            for d in range(DH):
                idx[pair * 128 + hh * 64 + d] = d * H + h_g
    return idx


def make_in_maps(from_tensor, to_tensor, mask, Wq, bq, Wk, bk, Wv, bv):
    per_b = {}
    for b in range(B):
        xTb = from_tensor[b].T.astype(bf16)     # [C, F]
        yTb = to_tensor[b].T.astype(bf16)       # [C, T]
        mTb = mask[b].T.astype(bf16)            # [T, F]
        per_b[b] = (
            np.ascontiguousarray(xTb.reshape(C, 4, 512).transpose(1, 0, 2)),
            np.ascontiguousarray(yTb.reshape(C, 4, 512).transpose(1, 0, 2)),
            np.ascontiguousarray(mTb.reshape(T, 4, 512).transpose(1, 0, 2)),
        )
    in_maps = []
    for i in range(NCORES):
        b, hg = i // 2, i % 2
        xTb, yTb, mTb = per_b[b]
        sl = slice(hg * COLS, (hg + 1) * COLS)
        kidx = _kperm(hg)
        in_maps.append(
            {
                "xT": xTb,
                "yT": yTb,
                "maskT": mTb,
                "wq": np.ascontiguousarray(Wq[:, sl]).astype(bf16),
                "wk": np.ascontiguousarray(Wk[:, kidx]).astype(bf16),
                "wq0": np.ascontiguousarray(Wq[:, sl][:, 0:128]).astype(bf16),
                "wk0": np.ascontiguousarray(Wk[:, kidx][:, 0:128]).astype(bf16),
                "wv": np.ascontiguousarray(Wv[:, sl]).astype(bf16),
                "bq": np.ascontiguousarray(
                    bq[sl].astype(np.float32).reshape(4, 128).T
                ),
                "bk": np.ascontiguousarray(
                    bk[kidx].astype(np.float32).reshape(4, 128).T
                ),
            }
        )
    return in_maps


def kernel(from_tensor, to_tensor, mask, Wq, bq, Wk, bk, Wv, bv):
    global LAST_RESULTS
    from_tensor = np.asarray(from_tensor, dtype=np.float32)
    to_tensor = np.asarray(to_tensor, dtype=np.float32)
    mask_np = np.asarray(mask)
    Wq = np.asarray(Wq, dtype=np.float32)
    Wk = np.asarray(Wk, dtype=np.float32)
    Wv = np.asarray(Wv, dtype=np.float32)
    bq = np.asarray(bq, dtype=np.float32)
    bk = np.asarray(bk, dtype=np.float32)
    bv = np.asarray(bv, dtype=np.float32)

    nc = _build()
    in_maps = make_in_maps(
        from_tensor, to_tensor, mask_np, Wq, bq, Wk, bk, Wv, bv
    )
    res = bass_utils.run_bass_kernel_spmd(
        nc, in_maps, core_ids=list(range(NCORES)), trace=PROFILE
    )
    LAST_RESULTS = res
    full = np.empty((B, F, H * DH), np.float32)
    for i in range(NCORES):
        b, hg = i // 2, i % 2
        # context bias is additive: ctx(v + bv) = ctx(v) + bv
        full[b, :, hg * COLS:(hg + 1) * COLS] = (
            res.results[i]["out"] + bv[hg * COLS:(hg + 1) * COLS]
        )
    return full


# revision 24
# speedup vs baseline: 1.1904x; 1.1762x over previous
"""Trainium2 Bass kernel for nn_Attention_12137577578573 (v4).

Full multi-head attention (QKV projection + masked softmax + context) for
B=4, F=T=2048, CF=CT=1024, H=16, DH=64, sharded over 8 NeuronCores as
(batch b, head-group hg): core i = (b = i // 2, hg = i % 2), each core
computing 1 batch x 8 heads.

Layout strategy (contraction dim on partitions):
  - host pre-transposes from/to tensors -> xT/yT [C, F]; yT and K^T live
    as per-512-column-chunk tiles so early chains don't dep-wait on whole
    tensors (Tile deps are tile-granular).
  - Q^T, K^T in transposed layout [cols, F]/[cols, T]; 2 heads per
    128-partition tile.
  - scores S^T [T, F] (T on partitions); softmax denominator via a
    ones-column appended to V; bv folded out on the host (context bias
    is additive); mask applied as P = exp(alpha*S) * maskT.
  - exp emission per tt ([128,1024] PSUM -> bf16 SBUF); mask multiplies
    are emitted one unit LATE (during unit u+1) so their DMA-wait can't
    head-of-line-block the DVE FIFO behind projection bias-adds.
  - context of unit u-1 runs tt-outer, 2 t-tiles per slot in slots 1-8 of
    unit u (consumes v[tt] incrementally, frees pT(u-1) by mid-unit so
    exp(u+1) never WAR-stalls on the pT pool).

The reference reshapes K as (T, DH, H) (head axis interleaved), unlike
Q/V (H, DH) — handled by a host-side column permutation of Wk/bk.
"""

import sys

if "/opt/trn_rl_repo" not in sys.path:
    sys.path.insert(0, "/opt/trn_rl_repo")

import numpy as np
import ml_dtypes

import concourse.bass as bass
import concourse.bacc as bacc
import concourse.mybir as mybir
import concourse.tile as tile
from concourse import bass_utils

BF16 = mybir.dt.bfloat16
F32 = mybir.dt.float32
bf16 = ml_dtypes.bfloat16

B, F, T, C, H, DH = 4, 2048, 2048, 1024, 16, 64
HL = 8          # heads per core
COLS = HL * DH  # 512 projected columns per core
ALPHA = 0.125   # 1/sqrt(64)
NCORES = 8
KT = C // 128   # 8 contraction tiles for projections
NFT = F // 128  # 16 F tiles
NTT = T // 128  # 16 T tiles
NPAIR = 4       # head pairs per core

# Toggled by test.py for profiling runs.
PROFILE = False
LAST_RESULTS = None

_nc_cache = None


def _emit(tc, nc, aps):
    xT, yT, maskT, wq, wk, wv, wq0d, wk0d, bq, bk, out = aps
    Exp = mybir.ActivationFunctionType.Exp

    import contextlib

    with contextlib.ExitStack() as ctx:
        pool = ctx.enter_context(tc.tile_pool(name="static", bufs=1))
        xTp = ctx.enter_context(tc.tile_pool(name="xTp", bufs=1))
        qTp = ctx.enter_context(tc.tile_pool(name="qTp", bufs=2))
        maskp = ctx.enter_context(tc.tile_pool(name="maskp", bufs=3))
        pTp = ctx.enter_context(tc.tile_pool(name="pTp", bufs=2))
        outp = ctx.enter_context(tc.tile_pool(name="outp", bufs=1))
        dinvp = ctx.enter_context(tc.tile_pool(name="dinvp", bufs=4))
        psum_s = ctx.enter_context(tc.tile_pool(name="psum_s", bufs=3, space="PSUM"))
        psum_ctx = ctx.enter_context(tc.tile_pool(name="psum_ctx", bufs=2, space="PSUM"))

        # Static tiles — yT and kT split per 512-col chunk for fine deps.
        kT4 = [[pool.tile([128, 512], BF16, name=f"kT{cb}_{c}", tag=f"kT{cb}_{c}")
                for c in range(4)] for cb in range(4)]
        yT4 = [[pool.tile([128, 512], BF16, name=f"yT{k}_{c}", tag=f"yT{k}_{c}")
                for c in range(4)] for k in range(KT)]
        v = [pool.tile([128, HL * 65], BF16, name=f"v{tt}", tag=f"v{tt}")
             for tt in range(NTT)]
        wq_sb = [pool.tile([128, COLS], BF16, name=f"wq{k}", tag=f"wq{k}") for k in range(KT)]
        wk_sb = [pool.tile([128, COLS], BF16, name=f"wk{k}", tag=f"wk{k}") for k in range(KT)]
        wq0 = [pool.tile([128, 128], BF16, name=f"wq0_{k}", tag=f"wq0_{k}") for k in range(KT)]
        wk0 = [pool.tile([128, 128], BF16, name=f"wk0_{k}", tag=f"wk0_{k}") for k in range(KT)]
        wv_sb = [pool.tile([128, COLS], BF16, name=f"wv{k}", tag=f"wv{k}") for k in range(KT)]
        bq_sb = pool.tile([128, 4], F32, name="bq_sb", tag="bq_sb")
        bk_sb = pool.tile([128, 4], F32, name="bk_sb", tag="bk_sb")

        xT_r = xT.rearrange("c (k p) f -> c p k f", p=128)
        maskT_r = maskT.rearrange("c (tt p) f -> c p tt f", p=128)
        out_r = out.rearrange("(g p) c -> p g c", p=128)

        # ---- upfront DMA queue (sync engine FIFO). Order: everything the
        # first exp needs (y chunk0, wk, wq, x chunk0) lands by ~12us.
        nc.sync.dma_start(bk_sb[:], bk[:])
        nc.sync.dma_start(bq_sb[:], bq[:])
        warm_sb = pool.tile([1, 8], F32, name="warm_sb", tag="warm_sb")
        nc.vector.memset(warm_sb[:], 0.0)
        nc.scalar.activation(warm_sb[:], warm_sb[:], Exp)

        # 4 parallel DMA queues (~190 GB/s each): sync=y0/y1, vector=wv/y2/y3,
        # scalar=fc0 masks, gpsimd=weights+x0. First-exp prereqs land ~10us.
        # critical path on sync; y1-3 stream in parallel on gpsimd
        for k in range(KT):
            nc.gpsimd.dma_start(yT4[k][1][:], yT[1, k * 128:(k + 1) * 128, :])
        for k in range(KT):
            nc.gpsimd.dma_start(yT4[k][2][:], yT[2, k * 128:(k + 1) * 128, :])
        for k in range(KT):
            nc.gpsimd.dma_start(yT4[k][3][:], yT[3, k * 128:(k + 1) * 128, :])
        for k in range(KT):
            nc.sync.dma_start(yT4[k][0][:], yT[0, k * 128:(k + 1) * 128, :])
        for k in range(KT):
            nc.sync.dma_start(wk0[k][:], wk0d[k * 128:(k + 1) * 128, :])
        for k in range(KT):
            nc.sync.dma_start(wq0[k][:], wq0d[k * 128:(k + 1) * 128, :])
        xTt = xTp.tile([128, KT, 512], BF16, name="xTt", tag="xT")
        nc.sync.dma_start(xTt[:], xT_r[0])
        for k in range(KT):
            nc.sync.dma_start(wk_sb[k][:], wk[k * 128:(k + 1) * 128, :])
        for k in range(KT):
            nc.sync.dma_start(wq_sb[k][:], wq[k * 128:(k + 1) * 128, :])
        for k in range(KT):
            nc.sync.dma_start(wv_sb[k][:], wv[k * 128:(k + 1) * 128, :])
        mask_h = {}
        for half in range(2):
            mh = maskp.tile([128, 8, 512], BF16, name="mh", tag="mask")
            nc.sync.dma_start(mh[:], maskT_r[0, :, half * 8:(half + 1) * 8, :])
            mask_h[(0, half)] = mh

        # ---- chain emitters (PE work units) ----
        def k_chain(cb, tcc):
            ps = psum_s.tile([128, 1024], F32, name="ps_s", tag="s")[:, 0:512]
            for k in range(KT):
                nc.tensor.matmul(
                    ps[:],
                    wk0[k][:] if cb == 0 else wk_sb[k][:, cb * 128:(cb + 1) * 128],
                    yT4[k][tcc][:],
                    start=(k == 0),
                    stop=(k == KT - 1),
                )
            nc.vector.tensor_scalar_add(kT4[cb][tcc][:], ps[:], bk_sb[:, cb:cb + 1])

        def v_chain(tt):
            ps = psum_s.tile([128, 1024], F32, name="ps_s", tag="s")[:, 0:512]
            for k in range(KT):
                nc.tensor.matmul(
                    ps[:],
                    yT4[k][tt // 4][:, (tt % 4) * 128:(tt % 4 + 1) * 128],
                    wv_sb[k][:],
                    start=(k == 0),
                    stop=(k == KT - 1),
                )
            vview = v[tt].rearrange("p (h c) -> p h c", c=65)
            nc.vector.tensor_copy(
                vview[:, :, 0:64], ps.rearrange("p (h c) -> p h c", c=64)[:]
            )
            nc.vector.memset(vview[:, :, 64:65], 1.0)

        qT_tiles = {}

        def q_chain(fc, cb, xt):
            qt = qT_tiles[fc]
            ps = psum_s.tile([128, 1024], F32, name="ps_s", tag="s")[:, 0:512]
            for k in range(KT):
                nc.tensor.matmul(
                    ps[:],
                    wq0[k][:] if cb == 0 else wq_sb[k][:, cb * 128:(cb + 1) * 128],
                    xt[:, k, :],
                    start=(k == 0),
                    stop=(k == KT - 1),
                )
            nc.vector.tensor_scalar_add(qt[:, cb, :], ps[:], bq_sb[:, cb:cb + 1])

        # ---- unit machinery ----
        pT_store = {}
        ctx_ps = {}
        out_tiles = {}

        def emit_scores_tt(u, tt):
            fc, pair = u // 4, u % 4
            qt = qT_tiles[fc]
            ps = psum_s.tile([128, 1024], F32, name="ps_s", tag="s")
            for hh in range(2):
                nc.tensor.matmul(
                    ps[:, hh * 512:(hh + 1) * 512],
                    kT4[pair][tt // 4][hh * 64:(hh + 1) * 64,
                                       (tt % 4) * 128:(tt % 4 + 1) * 128],
                    qt[hh * 64:(hh + 1) * 64, pair, :],
                    start=True, stop=True,
                )
            nc.scalar.activation(pT_store[u][:, tt, :], ps[:], Exp, scale=ALPHA)

        def emit_mask_4tt(u, tt0):
            fc = u // 4
            mh = mask_h[(fc, tt0 // 8)]
            o = pT_store[u][:, tt0:tt0 + 4, :].rearrange(
                "p t (h c) -> p t h c", c=512
            )
            m = mh[:, tt0 % 8: tt0 % 8 + 4, :].unsqueeze(2).broadcast_to(
                [128, 4, 2, 512]
            )
            nc.vector.tensor_mul(o[:], o[:], m)

        def emit_ctx_chain(cu, j, half=None):
            """Chain j in 0..7: (hh = j//4, ft = j%4); half=0/1 emits tt 0-7 /
            8-15. After each head's last chain, evacuate it."""
            pair = cu % 4
            hh, ft = j // 4, j % 4
            if ft == 0 and half in (None, 0):
                ctx_ps.setdefault(cu, {})[hh] = psum_ctx.tile(
                    [128, 512], F32, name="pc", tag="pc"
                )
            pc = ctx_ps[cu][hh]
            pt = pT_store[cu]
            h = pair * 2 + hh
            tts = range(NTT) if half is None else range(half * 8, half * 8 + 8)
            for tt in tts:
                nc.tensor.matmul(
                    pc[:, ft * 65:ft * 65 + 65],
                    pt[:, tt, hh * 512 + ft * 128: hh * 512 + (ft + 1) * 128],
                    v[tt][:, h * 65:(h + 1) * 65],
                    start=(tt == 0),
                    stop=(tt == NTT - 1),
                )
            if ft == 3 and half in (None, 1):
                emit_ctx_evac_h(cu, hh)

        def emit_ctx_evac_h(cu, hh):
            fc, pair = cu // 4, cu % 4
            if cu not in out_tiles:
                out_tiles[cu] = outp.tile([128, 4, 128], F32, name="outt", tag="out")
            ot = out_tiles[cu]
            pc = ctx_ps[cu][hh]
            dinv = dinvp.tile([128, 4], F32, name="dinv", tag="dinv")
            nc.vector.reciprocal(
                dinv.rearrange("p (a b) -> p a b", b=1)[:],
                pc[:, 0:260].rearrange("p (ft c) -> p ft c", c=65)[:, :, 64:65],
            )
            nc.vector.tensor_mul(
                ot[:, :, hh * 64:(hh + 1) * 64],
                pc[:, 0:260].rearrange("p (ft c) -> p ft c", c=65)[:, :, 0:64],
                dinv.unsqueeze(2).broadcast_to([128, 4, 64]),
            )
            if hh == 1:
                ctx_ps.pop(cu)
                del pT_store[cu]
                nc.gpsimd.dma_start(
                    out_r[:, fc * 4:(fc + 1) * 4, pair * 128:(pair + 1) * 128],
                    ot[:],
                )
                del out_tiles[cu]

        def unit(u, fillers, ctx_u, dmas=(), late_dmas=(), ctx_late=False,
                 late_masks=False):
            for d in dmas:
                d()
            pT_store[u] = pTp.tile([128, NTT, 1024], BF16, name="pT", tag="pT")
            nf = len(fillers)
            fspan = 8 if ctx_late else NTT
            fi = 0
            for tt in range(NTT):
                emit_scores_tt(u, tt)
                if late_masks:
                    # all 4 mask ops at slots 12-15: keeps early DVE FIFO
                    # free of mask-DMA waits during the startup units
                    if tt >= 12:
                        emit_mask_4tt(u, 4 * (tt - 12))
                elif tt % 4 == 3:
                    emit_mask_4tt(u, tt - 3)
                want = nf if tt >= fspan else (tt + 1) * nf // fspan
                while fi < want:
                    fillers[fi]()
                    fi += 1
                if ctx_u is not None:
                    if ctx_late and tt >= 8:
                        emit_ctx_chain(ctx_u, tt - 8)
                    elif not ctx_late:
                        emit_ctx_chain(ctx_u, tt // 2, half=tt % 2)
                if tt == 10:
                    for d in late_dmas:
                        d()

        # deferred DMA emitters
        def dma_xt(fc):
            def go():
                xt = xTp.tile([128, KT, 512], BF16, name="xTt", tag="xT")
                nc.sync.dma_start(xt[:], xT_r[fc])
                dma_xt.tiles[fc] = xt
            return go
        dma_xt.tiles = {0: xTt}

        def dma_mask(fc, half):
            def go():
                mh = maskp.tile([128, 8, 512], BF16, name="mh", tag="mask")
                nc.sync.dma_start(
                    mh[:], maskT_r[fc, :, half * 8:(half + 1) * 8, :]
                )
                mask_h[(fc, half)] = mh
            return go

        # ---- prologue ----
        qT_tiles[0] = qTp.tile([128, 4, 512], BF16, name="qTt", tag="qT")
        k_chain(0, 0)
        q_chain(0, 0, xTt)

        def q_fillers(fc):
            qT_tiles[fc] = qTp.tile([128, 4, 512], BF16, name="qTt", tag="qT")
            return [
                (lambda cb=cb: q_chain(fc, cb, dma_xt.tiles[fc])) for cb in range(4)
            ]

        # ---- 16 units ----
        unit(0, [lambda: k_chain(0, 1), lambda: k_chain(0, 2),
                 lambda: k_chain(0, 3), lambda: k_chain(1, 0),
                 lambda: k_chain(1, 1), lambda: k_chain(1, 2),
                 lambda: k_chain(1, 3)]
                + [lambda t=t: v_chain(t) for t in range(4)]
                + [lambda: q_chain(0, 1, xTt)],
             None, dmas=(dma_xt(1),), late_masks=True)
        unit(1, [lambda t=t: v_chain(t) for t in range(4, 16)]
                + [lambda t=t: k_chain(2, t) for t in range(4)]
                + [lambda: q_chain(0, 2, xTt)],
             0, ctx_late=True, late_masks=True)
        unit(2, [lambda t=t: k_chain(3, t) for t in range(4)]
                + [lambda: q_chain(0, 3, xTt)], 1)
        unit(3, q_fillers(1), 2, late_dmas=(dma_mask(1, 0), dma_mask(1, 1)))
        unit(4, [], 3)
        unit(5, [], 4, dmas=(dma_xt(2),))
        unit(6, [], 5)
        unit(7, q_fillers(2), 6, late_dmas=(dma_mask(2, 0), dma_mask(2, 1)))
        unit(8, [], 7)
        unit(9, [], 8, dmas=(dma_xt(3),))
        unit(10, [], 9)
        unit(11, q_fillers(3), 10, late_dmas=(dma_mask(3, 0), dma_mask(3, 1)))
        unit(12, [], 11)
        unit(13, [], 12)
        unit(14, [], 13)
        unit(15, [], 14)
        # tail: context of the last unit
        for j in range(8):
            emit_ctx_chain(15, j)


def _build():
    global _nc_cache
    if _nc_cache is not None:
        return _nc_cache
    nc = bacc.Bacc(
        "TRN2",
        target_bir_lowering=False,
        debug=False,
        enable_asserts=False,
        num_devices=NCORES,
    )
    xT = nc.dram_tensor("xT", [4, C, 512], BF16, kind="ExternalInput").ap()
    yT = nc.dram_tensor("yT", [4, C, 512], BF16, kind="ExternalInput").ap()
    maskT = nc.dram_tensor("maskT", [4, T, 512], BF16, kind="ExternalInput").ap()
    wq = nc.dram_tensor("wq", [C, COLS], BF16, kind="ExternalInput").ap()
    wk = nc.dram_tensor("wk", [C, COLS], BF16, kind="ExternalInput").ap()
    wq0d = nc.dram_tensor("wq0", [C, 128], BF16, kind="ExternalInput").ap()
    wk0d = nc.dram_tensor("wk0", [C, 128], BF16, kind="ExternalInput").ap()
    wv = nc.dram_tensor("wv", [C, COLS], BF16, kind="ExternalInput").ap()
    bq = nc.dram_tensor("bq", [128, 4], F32, kind="ExternalInput").ap()
    bk = nc.dram_tensor("bk", [128, 4], F32, kind="ExternalInput").ap()
    out = nc.dram_tensor("out", [F, COLS], F32, kind="ExternalOutput").ap()

    with tile.TileContext(nc) as tc:
        _emit(tc, nc, (xT, yT, maskT, wq, wk, wv, wq0d, wk0d, bq, bk, out))
    nc.compile()
    _nc_cache = nc
    return nc


def _kperm(hg):
    """Local K column (pair*128 + hh*64 + d) -> global Wk column d*H + h_g."""
    idx = np.empty(COLS, dtype=np.int64)
    for pair in range(NPAIR):
        for hh in range(2):
            h_g = hg * HL + pair * 2 + hh
            for d in range(DH):
                idx[pair * 128 + hh * 64 + d] = d * H + h_g
    return idx


def make_in_maps(from_tensor, to_tensor, mask, Wq, bq, Wk, bk, Wv, bv):
    per_b = {}
    for b in range(B):
        xTb = from_tensor[b].T.astype(bf16)     # [C, F]
        yTb = to_tensor[b].T.astype(bf16)       # [C, T]
        mTb = mask[b].T.astype(bf16)            # [T, F]
        per_b[b] = (
            np.ascontiguousarray(xTb.reshape(C, 4, 512).transpose(1, 0, 2)),
            np.ascontiguousarray(yTb.reshape(C, 4, 512).transpose(1, 0, 2)),
            np.ascontiguousarray(mTb.reshape(T, 4, 512).transpose(1, 0, 2)),
        )
    in_maps = []
    for i in range(NCORES):
        b, hg = i // 2, i % 2
        xTb, yTb, mTb = per_b[b]
        sl = slice(hg * COLS, (hg + 1) * COLS)
        kidx = _kperm(hg)
        in_maps.append(
            {
                "xT": xTb,
                "yT": yTb,
                "maskT": mTb,
                "wq": np.ascontiguousarray(Wq[:, sl]).astype(bf16),
                "wk": np.ascontiguousarray(Wk[:, kidx]).astype(bf16),
                "wq0": np.ascontiguousarray(Wq[:, sl][:, 0:128]).astype(bf16),
                "wk0": np.ascontiguousarray(Wk[:, kidx][:, 0:128]).astype(bf16),
                "wv": np.ascontiguousarray(Wv[:, sl]).astype(bf16),
                "bq": np.ascontiguousarray(
                    bq[sl].astype(np.float32).reshape(4, 128).T
                ),
                "bk": np.ascontiguousarray(
                    bk[kidx].astype(np.float32).reshape(4, 128).T
                ),
            }
        )
    return in_maps


def kernel(from_tensor, to_tensor, mask, Wq, bq, Wk, bk, Wv, bv):
    global LAST_RESULTS
    from_tensor = np.asarray(from_tensor, dtype=np.float32)
    to_tensor = np.asarray(to_tensor, dtype=np.float32)
    mask_np = np.asarray(mask)
    Wq = np.asarray(Wq, dtype=np.float32)
    Wk = np.asarray(Wk, dtype=np.float32)
    Wv = np.asarray(Wv, dtype=np.float32)
    bq = np.asarray(bq, dtype=np.float32)
    bk = np.asarray(bk, dtype=np.float32)
    bv = np.asarray(bv, dtype=np.float32)

    nc = _build()
    in_maps = make_in_maps(
        from_tensor, to_tensor, mask_np, Wq, bq, Wk, bk, Wv, bv
    )
    res = bass_utils.run_bass_kernel_spmd(
        nc, in_maps, core_ids=list(range(NCORES)), trace=PROFILE
    )
    LAST_RESULTS = res
    full = np.empty((B, F, H * DH), np.float32)
    for i in range(NCORES):
        b, hg = i // 2, i % 2
        # context bias is additive: ctx(v + bv) = ctx(v) + bv
        full[b, :, hg * COLS:(hg + 1) * COLS] = (
            res.results[i]["out"] + bv[hg * COLS:(hg + 1) * COLS]
        )
    return full


# revision 25
# speedup vs baseline: 1.1955x; 1.0043x over previous
"""Trainium2 Bass kernel for nn_Attention_12137577578573 (v4).

Full multi-head attention (QKV projection + masked softmax + context) for
B=4, F=T=2048, CF=CT=1024, H=16, DH=64, sharded over 8 NeuronCores as
(batch b, head-group hg): core i = (b = i // 2, hg = i % 2), each core
computing 1 batch x 8 heads.

Layout strategy (contraction dim on partitions):
  - host pre-transposes from/to tensors -> xT/yT [C, F]; yT and K^T live
    as per-512-column-chunk tiles so early chains don't dep-wait on whole
    tensors (Tile deps are tile-granular).
  - Q^T, K^T in transposed layout [cols, F]/[cols, T]; 2 heads per
    128-partition tile.
  - scores S^T [T, F] (T on partitions); softmax denominator via a
    ones-column appended to V; bv folded out on the host (context bias
    is additive); mask applied as P = exp(alpha*S) * maskT.
  - exp emission per tt ([128,1024] PSUM -> bf16 SBUF); mask multiplies
    are emitted one unit LATE (during unit u+1) so their DMA-wait can't
    head-of-line-block the DVE FIFO behind projection bias-adds.
  - context of unit u-1 runs tt-outer, 2 t-tiles per slot in slots 1-8 of
    unit u (consumes v[tt] incrementally, frees pT(u-1) by mid-unit so
    exp(u+1) never WAR-stalls on the pT pool).

The reference reshapes K as (T, DH, H) (head axis interleaved), unlike
Q/V (H, DH) — handled by a host-side column permutation of Wk/bk.
"""

import sys

if "/opt/trn_rl_repo" not in sys.path:
    sys.path.insert(0, "/opt/trn_rl_repo")

import numpy as np
import ml_dtypes

import concourse.bass as bass
import concourse.bacc as bacc
import concourse.mybir as mybir
import concourse.tile as tile
from concourse import bass_utils

BF16 = mybir.dt.bfloat16
F32 = mybir.dt.float32
bf16 = ml_dtypes.bfloat16

B, F, T, C, H, DH = 4, 2048, 2048, 1024, 16, 64
HL = 8          # heads per core
COLS = HL * DH  # 512 projected columns per core
ALPHA = 0.125   # 1/sqrt(64)
NCORES = 8
KT = C // 128   # 8 contraction tiles for projections
NFT = F // 128  # 16 F tiles
NTT = T // 128  # 16 T tiles
NPAIR = 4       # head pairs per core

# Toggled by test.py for profiling runs.
PROFILE = False
LAST_RESULTS = None

_nc_cache = None


def _emit(tc, nc, aps):
    xT, yT, maskT, wq, wk, wv, bq, bk, out = aps
    Exp = mybir.ActivationFunctionType.Exp

    import contextlib

    with contextlib.ExitStack() as ctx:
        pool = ctx.enter_context(tc.tile_pool(name="static", bufs=1))
        xTp = ctx.enter_context(tc.tile_pool(name="xTp", bufs=1))
        qTp = ctx.enter_context(tc.tile_pool(name="qTp", bufs=2))
        maskp = ctx.enter_context(tc.tile_pool(name="maskp", bufs=3))
        pTp = ctx.enter_context(tc.tile_pool(name="pTp", bufs=2))
        outp = ctx.enter_context(tc.tile_pool(name="outp", bufs=1))
        dinvp = ctx.enter_context(tc.tile_pool(name="dinvp", bufs=4))
        psum_s = ctx.enter_context(tc.tile_pool(name="psum_s", bufs=3, space="PSUM"))
        psum_ctx = ctx.enter_context(tc.tile_pool(name="psum_ctx", bufs=2, space="PSUM"))

        # Static tiles — yT and kT split per 512-col chunk for fine deps.
        kT4 = [[pool.tile([128, 512], BF16, name=f"kT{cb}_{c}", tag=f"kT{cb}_{c}")
                for c in range(4)] for cb in range(4)]
        yT4 = [[pool.tile([128, 512], BF16, name=f"yT{k}_{c}", tag=f"yT{k}_{c}")
                for c in range(4)] for k in range(KT)]
        v = [pool.tile([128, HL * 65], BF16, name=f"v{tt}", tag=f"v{tt}")
             for tt in range(NTT)]
        wq_sb = [pool.tile([128, COLS], BF16, name=f"wq{k}", tag=f"wq{k}") for k in range(KT)]
        wk_sb = [pool.tile([128, COLS], BF16, name=f"wk{k}", tag=f"wk{k}") for k in range(KT)]
        wv_sb = [pool.tile([128, COLS], BF16, name=f"wv{k}", tag=f"wv{k}") for k in range(KT)]
        bq_sb = pool.tile([128, 4], F32, name="bq_sb", tag="bq_sb")
        bk_sb = pool.tile([128, 4], F32, name="bk_sb", tag="bk_sb")

        xT_r = xT.rearrange("c (k p) f -> c p k f", p=128)
        maskT_r = maskT.rearrange("c (tt p) f -> c p tt f", p=128)
        out_r = out.rearrange("(g p) c -> p g c", p=128)

        # ---- upfront DMA queue (sync engine FIFO). Order: everything the
        # first exp needs (y chunk0, wk, wq, x chunk0) lands by ~12us.
        nc.sync.dma_start(bk_sb[:], bk[:])
        nc.sync.dma_start(bq_sb[:], bq[:])
        warm_sb = pool.tile([1, 8], F32, name="warm_sb", tag="warm_sb")
        nc.vector.memset(warm_sb[:], 0.0)
        nc.scalar.activation(warm_sb[:], warm_sb[:], Exp)

        # 4 parallel DMA queues (~190 GB/s each): sync=y0/y1, vector=wv/y2/y3,
        # scalar=fc0 masks, gpsimd=weights+x0. First-exp prereqs land ~10us.
        for k in range(KT):
            nc.sync.dma_start(yT4[k][0][:], yT[0, k * 128:(k + 1) * 128, :])
        for k in range(KT):
            nc.sync.dma_start(wk_sb[k][:], wk[k * 128:(k + 1) * 128, :])
        for k in range(KT):
            nc.sync.dma_start(wq_sb[k][:], wq[k * 128:(k + 1) * 128, :])
        xTt = xTp.tile([128, KT, 512], BF16, name="xTt", tag="xT")
        nc.sync.dma_start(xTt[:], xT_r[0])
        for c in range(1, 4):
            for k in range(KT):
                nc.sync.dma_start(yT4[k][c][:], yT[c, k * 128:(k + 1) * 128, :])
        for k in range(KT):
            nc.sync.dma_start(wv_sb[k][:], wv[k * 128:(k + 1) * 128, :])
        mask_h = {}
        for half in range(2):
            mh = maskp.tile([128, 8, 512], BF16, name="mh", tag="mask")
            nc.sync.dma_start(mh[:], maskT_r[0, :, half * 8:(half + 1) * 8, :])
            mask_h[(0, half)] = mh

        # ---- chain emitters (PE work units) ----
        def k_chain(cb, tcc):
            ps = psum_s.tile([128, 1024], F32, name="ps_s", tag="s")[:, 0:512]
            for k in range(KT):
                nc.tensor.matmul(
                    ps[:],
                    wk_sb[k][:, cb * 128:(cb + 1) * 128],
                    yT4[k][tcc][:],
                    start=(k == 0),
                    stop=(k == KT - 1),
                )
            nc.vector.tensor_scalar_add(kT4[cb][tcc][:], ps[:], bk_sb[:, cb:cb + 1])

        def v_chain(tt):
            ps = psum_s.tile([128, 1024], F32, name="ps_s", tag="s")[:, 0:512]
            for k in range(KT):
                nc.tensor.matmul(
                    ps[:],
                    yT4[k][tt // 4][:, (tt % 4) * 128:(tt % 4 + 1) * 128],
                    wv_sb[k][:],
                    start=(k == 0),
                    stop=(k == KT - 1),
                )
            vview = v[tt].rearrange("p (h c) -> p h c", c=65)
            nc.vector.tensor_copy(
                vview[:, :, 0:64], ps.rearrange("p (h c) -> p h c", c=64)[:]
            )
            nc.vector.memset(vview[:, :, 64:65], 1.0)

        qT_tiles = {}

        def q_chain(fc, cb, xt):
            qt = qT_tiles[fc]
            ps = psum_s.tile([128, 1024], F32, name="ps_s", tag="s")[:, 0:512]
            for k in range(KT):
                nc.tensor.matmul(
                    ps[:],
                    wq_sb[k][:, cb * 128:(cb + 1) * 128],
                    xt[:, k, :],
                    start=(k == 0),
                    stop=(k == KT - 1),
                )
            nc.vector.tensor_scalar_add(qt[:, cb, :], ps[:], bq_sb[:, cb:cb + 1])

        # ---- unit machinery ----
        pT_store = {}
        ctx_ps = {}
        out_tiles = {}

        def emit_scores_tt(u, tt):
            fc, pair = u // 4, u % 4
            qt = qT_tiles[fc]
            ps = psum_s.tile([128, 1024], F32, name="ps_s", tag="s")
            for hh in range(2):
                nc.tensor.matmul(
                    ps[:, hh * 512:(hh + 1) * 512],
                    kT4[pair][tt // 4][hh * 64:(hh + 1) * 64,
                                       (tt % 4) * 128:(tt % 4 + 1) * 128],
                    qt[hh * 64:(hh + 1) * 64, pair, :],
                    start=True, stop=True,
                )
            nc.scalar.activation(pT_store[u][:, tt, :], ps[:], Exp, scale=ALPHA)

        def emit_mask_4tt(u, tt0):
            fc = u // 4
            mh = mask_h[(fc, tt0 // 8)]
            o = pT_store[u][:, tt0:tt0 + 4, :].rearrange(
                "p t (h c) -> p t h c", c=512
            )
            m = mh[:, tt0 % 8: tt0 % 8 + 4, :].unsqueeze(2).broadcast_to(
                [128, 4, 2, 512]
            )
            nc.vector.tensor_mul(o[:], o[:], m)

        def emit_ctx_chain(cu, j, half=None):
            """Chain j in 0..7: (hh = j//4, ft = j%4); half=0/1 emits tt 0-7 /
            8-15. After each head's last chain, evacuate it."""
            pair = cu % 4
            hh, ft = j // 4, j % 4
            if ft == 0 and half in (None, 0):
                ctx_ps.setdefault(cu, {})[hh] = psum_ctx.tile(
                    [128, 512], F32, name="pc", tag="pc"
                )
            pc = ctx_ps[cu][hh]
            pt = pT_store[cu]
            h = pair * 2 + hh
            tts = range(NTT) if half is None else range(half * 8, half * 8 + 8)
            for tt in tts:
                nc.tensor.matmul(
                    pc[:, ft * 65:ft * 65 + 65],
                    pt[:, tt, hh * 512 + ft * 128: hh * 512 + (ft + 1) * 128],
                    v[tt][:, h * 65:(h + 1) * 65],
                    start=(tt == 0),
                    stop=(tt == NTT - 1),
                )
            if ft == 3 and half in (None, 1):
                emit_ctx_evac_h(cu, hh)

        def emit_ctx_evac_h(cu, hh):
            fc, pair = cu // 4, cu % 4
            if cu not in out_tiles:
                out_tiles[cu] = outp.tile([128, 4, 128], F32, name="outt", tag="out")
            ot = out_tiles[cu]
            pc = ctx_ps[cu][hh]
            dinv = dinvp.tile([128, 4], F32, name="dinv", tag="dinv")
            nc.vector.reciprocal(
                dinv.rearrange("p (a b) -> p a b", b=1)[:],
                pc[:, 0:260].rearrange("p (ft c) -> p ft c", c=65)[:, :, 64:65],
            )
            nc.vector.tensor_mul(
                ot[:, :, hh * 64:(hh + 1) * 64],
                pc[:, 0:260].rearrange("p (ft c) -> p ft c", c=65)[:, :, 0:64],
                dinv.unsqueeze(2).broadcast_to([128, 4, 64]),
            )
            if hh == 1:
                ctx_ps.pop(cu)
                del pT_store[cu]
                nc.gpsimd.dma_start(
                    out_r[:, fc * 4:(fc + 1) * 4, pair * 128:(pair + 1) * 128],
                    ot[:],
                )
                del out_tiles[cu]

        def unit(u, fillers, ctx_u, dmas=(), late_dmas=(), ctx_late=False):
            for d in dmas:
                d()
            pT_store[u] = pTp.tile([128, NTT, 1024], BF16, name="pT", tag="pT")
            nf = len(fillers)
            fspan = 8 if ctx_late else NTT
            fi = 0
            for tt in range(NTT):
                emit_scores_tt(u, tt)
                if tt % 4 == 3:
                    emit_mask_4tt(u, tt - 3)
                want = nf if tt >= fspan else (tt + 1) * nf // fspan
                while fi < want:
                    fillers[fi]()
                    fi += 1
                if ctx_u is not None:
                    if ctx_late and tt >= 8:
                        emit_ctx_chain(ctx_u, tt - 8)
                    elif not ctx_late:
                        emit_ctx_chain(ctx_u, tt // 2, half=tt % 2)
                if tt == 10:
                    for d in late_dmas:
                        d()

        # deferred DMA emitters
        def dma_xt(fc):
            def go():
                xt = xTp.tile([128, KT, 512], BF16, name="xTt", tag="xT")
                nc.sync.dma_start(xt[:], xT_r[fc])
                dma_xt.tiles[fc] = xt
            return go
        dma_xt.tiles = {0: xTt}

        def dma_mask(fc, half):
            def go():
                mh = maskp.tile([128, 8, 512], BF16, name="mh", tag="mask")
                nc.sync.dma_start(
                    mh[:], maskT_r[fc, :, half * 8:(half + 1) * 8, :]
                )
                mask_h[(fc, half)] = mh
            return go

        # ---- prologue ----
        qT_tiles[0] = qTp.tile([128, 4, 512], BF16, name="qTt", tag="qT")
        k_chain(0, 0)
        q_chain(0, 0, xTt)

        def q_fillers(fc):
            qT_tiles[fc] = qTp.tile([128, 4, 512], BF16, name="qTt", tag="qT")
            return [
                (lambda cb=cb: q_chain(fc, cb, dma_xt.tiles[fc])) for cb in range(4)
            ]

        # ---- 16 units ----
        unit(0, [lambda: k_chain(0, 1), lambda: k_chain(1, 0),
                 lambda: q_chain(0, 1, xTt), lambda: k_chain(0, 2),
                 lambda: k_chain(1, 1), lambda: k_chain(0, 3),
                 lambda: k_chain(1, 2), lambda: k_chain(1, 3)]
                + [lambda t=t: v_chain(t) for t in range(4)],
             None, dmas=(dma_xt(1),))
        unit(1, [lambda t=t: v_chain(t) for t in range(4, 16)]
                + [lambda t=t: k_chain(2, t) for t in range(4)]
                + [lambda: q_chain(0, 2, xTt)],
             0, ctx_late=True)
        unit(2, [lambda t=t: k_chain(3, t) for t in range(4)]
                + [lambda: q_chain(0, 3, xTt)], 1)
        unit(3, q_fillers(1), 2, late_dmas=(dma_mask(1, 0), dma_mask(1, 1)))
        unit(4, [], 3)
        unit(5, [], 4, dmas=(dma_xt(2),))
        unit(6, [], 5)
        unit(7, q_fillers(2), 6, late_dmas=(dma_mask(2, 0), dma_mask(2, 1)))
        unit(8, [], 7)
        unit(9, [], 8, dmas=(dma_xt(3),))
        unit(10, [], 9)
        unit(11, q_fillers(3), 10, late_dmas=(dma_mask(3, 0), dma_mask(3, 1)))
        unit(12, [], 11)
        unit(13, [], 12)
        unit(14, [], 13)
        unit(15, [], 14)
        # tail: context of the last unit
        for j in range(8):
            emit_ctx_chain(15, j)


def _build():
    global _nc_cache
    if _nc_cache is not None:
        return _nc_cache
    nc = bacc.Bacc(
        "TRN2",
        target_bir_lowering=False,
        debug=False,
        enable_asserts=False,
        num_devices=NCORES,
    )
    xT = nc.dram_tensor("xT", [4, C, 512], BF16, kind="ExternalInput").ap()
    yT = nc.dram_tensor("yT", [4, C, 512], BF16, kind="ExternalInput").ap()
    maskT = nc.dram_tensor("maskT", [4, T, 512], BF16, kind="ExternalInput").ap()
    wq = nc.dram_tensor("wq", [C, COLS], BF16, kind="ExternalInput").ap()
    wk = nc.dram_tensor("wk", [C, COLS], BF16, kind="ExternalInput").ap()
    wv = nc.dram_tensor("wv", [C, COLS], BF16, kind="ExternalInput").ap()
    bq = nc.dram_tensor("bq", [128, 4], F32, kind="ExternalInput").ap()
    bk = nc.dram_tensor("bk", [128, 4], F32, kind="ExternalInput").ap()
    out = nc.dram_tensor("out", [F, COLS], F32, kind="ExternalOutput").ap()

    with tile.TileContext(nc) as tc:
        _emit(tc, nc, (xT, yT, maskT, wq, wk, wv, bq, bk, out))
    nc.compile()
    _nc_cache = nc
    return nc


def _kperm(hg):
    """Local K column (pair*128 + hh*64 + d) -> global Wk column d*H + h_g."""
    idx = np.empty(COLS, dtype=np.int64)
    for pair in range(NPAIR):
        for hh in range(2):
            h_g = hg * HL + pair * 2 + hh
            for d in range(DH):
                idx[pair * 128 + hh * 64 + d] = d * H + h_g
    return idx


def make_in_maps(from_tensor, to_tensor, mask, Wq, bq, Wk, bk, Wv, bv):
    per_b = {}
    for b in range(B):
        xTb = from_tensor[b].T.astype(bf16)     # [C, F]
        yTb = to_tensor[b].T.astype(bf16)       # [C, T]
        mTb = mask[b].T.astype(bf16)            # [T, F]
        per_b[b] = (
            np.ascontiguousarray(xTb.reshape(C, 4, 512).transpose(1, 0, 2)),
            np.ascontiguousarray(yTb.reshape(C, 4, 512).transpose(1, 0, 2)),
            np.ascontiguousarray(mTb.reshape(T, 4, 512).transpose(1, 0, 2)),
        )
    in_maps = []
    for i in range(NCORES):
        b, hg = i // 2, i % 2
        xTb, yTb, mTb = per_b[b]
        sl = slice(hg * COLS, (hg + 1) * COLS)
        kidx = _kperm(hg)
        in_maps.append(
            {
                "xT": xTb,
                "yT": yTb,
                "maskT": mTb,
                "wq": np.ascontiguousarray(Wq[:, sl]).astype(bf16),
                "wk": np.ascontiguousarray(Wk[:, kidx]).astype(bf16),
                "wv": np.ascontiguousarray(Wv[:, sl]).astype(bf16),
                "bq": np.ascontiguousarray(
                    bq[sl].astype(np.float32).reshape(4, 128).T
                ),
                "bk": np.ascontiguousarray(
                    bk[kidx].astype(np.float32).reshape(4, 128).T
                ),
            }
        )
    return in_maps


def kernel(from_tensor, to_tensor, mask, Wq, bq, Wk, bk, Wv, bv):
    global LAST_RESULTS
    from_tensor = np.asarray(from_tensor, dtype=np.float32)
    to_tensor = np.asarray(to_tensor, dtype=np.float32)
    mask_np = np.asarray(mask)
    Wq = np.asarray(Wq, dtype=np.float32)
    Wk = np.asarray(Wk, dtype=np.float32)
    Wv = np.asarray(Wv, dtype=np.float32)
    bq = np.asarray(bq, dtype=np.float32)
    bk = np.asarray(bk, dtype=np.float32)
    bv = np.asarray(bv, dtype=np.float32)

    nc = _build()
    in_maps = make_in_maps(
        from_tensor, to_tensor, mask_np, Wq, bq, Wk, bk, Wv, bv
    )
    res = bass_utils.run_bass_kernel_spmd(
        nc, in_maps, core_ids=list(range(NCORES)), trace=PROFILE
    )
    LAST_RESULTS = res
    full = np.empty((B, F, H * DH), np.float32)
    for i in range(NCORES):
        b, hg = i // 2, i % 2
        # context bias is additive: ctx(v + bv) = ctx(v) + bv
        full[b, :, hg * COLS:(hg + 1) * COLS] = (
            res.results[i]["out"] + bv[hg * COLS:(hg + 1) * COLS]
        )
    return full
